# revision 2
# baseline (speedup 1.0000x reference)
"""Trainium2 Bass kernel for nn_BinarizeLayer (histogram_binning).

out[b, f] = (medians[f] > 0) & (inputs[b, f] >= medians[f])

Strategy (data parallel, memory-bound):
  - Shard the batch dim (8192) across 8 NeuronCores -> 1024 rows/core.
  - Host-side fold the (medians > 0) condition into a threshold vector:
        t[f] = medians[f] if medians[f] > 0 else +inf
    so the device does a single elementwise compare per tile:
        out = (x >= t_bcast)  (uint8 0/1, viewed as bool on the host)
  - t is DMA-broadcast once to all 128 partitions; the per-core shard is
    processed as 8 tiles of [128, 4096] f32 (each a contiguous 2 MiB DMA).
  - Output is written as uint8 (1 byte/elem) to quarter the write traffic
    vs f32, matching np.bool_'s memory layout.
"""

import json

import numpy as np

import concourse.bass as bass
import concourse.mybir as mybir
import concourse.bass_utils as _bass_utils
import concourse.bass2jax as _bass2jax
from concourse.tile import TileContext
from concourse.bass_utils import run_bass_kernel_spmd

B, F = 8192, 4096
NCORES = 8
ROWS_PER_CORE = B // NCORES  # 1024
P = 128
TILES_PER_CORE = ROWS_PER_CORE // P  # 8

# ---------------------------------------------------------------------------
# Workaround for the pinned walrus codegen: CoreV3 encodes at most ONE sem
# wait per instruction ("Too many sync wait commands"), but Tile's sem
# assignment attaches one wait per outstanding dependency to a single
# instruction. Rewrite the BIR before compiling: hoist all-but-one wait of
# any multi-wait instruction onto EventSemaphore carriers inserted just
# before it on the same engine (engines execute in order, so the combined
# wait set is identical).


def _split_multiwait_bir(bir_json) -> bytes:
    d = json.loads(bir_json)
    n_split = 0
    for fn in d.get("functions", []):
        for blk in fn.get("blocks", []):
            insts = blk.get("instructions")
            if not insts:
                continue
            out = []
            for ins in insts:
                si = ins.get("sync_info")
                waits = (si or {}).get("on_wait") or []
                if len(waits) > 1:
                    for w in waits[:-1]:
                        out.append(
                            {
                                "name": f"{ins['name']}-sw{n_split}",
                                "opcode": "EventSemaphore",
                                "engine": ins["engine"],
                                "ins": [],
                                "outs": [],
                                "debug": ins.get("debug"),
                                "sync_info": {"on_wait": [w], "on_update": []},
                            }
                        )
                        n_split += 1
                    si["on_wait"] = [waits[-1]]
                out.append(ins)
            blk["instructions"] = out
    return json.dumps(d).encode()


_orig_compile_bir_kernel = _bass_utils.compile_bir_kernel


def _patched_compile_bir_kernel(bir_json, tmpdir, neff_name="file.neff"):
    return _orig_compile_bir_kernel(
        _split_multiwait_bir(bir_json), tmpdir, neff_name
    )


if _bass_utils.compile_bir_kernel is not _patched_compile_bir_kernel:
    _bass_utils.compile_bir_kernel = _patched_compile_bir_kernel
    _bass2jax.compile_bir_kernel = _patched_compile_bir_kernel
# ---------------------------------------------------------------------------

TRACE = False  # test harness can flip this to collect an NTFF trace
LAST_RESULTS = None  # BassKernelResults of the most recent run (for timing)

_nc_cache = None


def _build_program():
    global _nc_cache
    if _nc_cache is not None:
        return _nc_cache

    nc = bass.Bass("TRN2", target_bir_lowering=False, debug=False,
                   num_devices=NCORES)
    x = nc.dram_tensor(
        "x", [ROWS_PER_CORE, F], mybir.dt.float32, kind="ExternalInput"
    ).ap()
    thr = nc.dram_tensor("thr", [1, F], mybir.dt.float32,
                         kind="ExternalInput").ap()
    out = nc.dram_tensor(
        "out", [ROWS_PER_CORE, F], mybir.dt.uint8, kind="ExternalOutput"
    ).ap()

    thr_bcast_ap = bass.AP(
        tensor=thr.tensor, offset=thr.offset, ap=[[0, P], thr.ap[1]]
    )

    with TileContext(nc) as tc:
        with tc.tile_pool(name="const", bufs=1) as const_pool, \
             tc.tile_pool(name="xin", bufs=3) as xin_pool, \
             tc.tile_pool(name="yout", bufs=3) as yout_pool:
            t_bcast = const_pool.tile([P, F], mybir.dt.float32)
            nc.gpsimd.dma_start(out=t_bcast, in_=thr_bcast_ap)

            for i in range(TILES_PER_CORE):
                xt = xin_pool.tile([P, F], mybir.dt.float32)
                nc.sync.dma_start(out=xt, in_=x[i * P:(i + 1) * P, :])
                ot = yout_pool.tile([P, F], mybir.dt.uint8)
                nc.vector.tensor_tensor(
                    out=ot, in0=xt, in1=t_bcast, op=mybir.AluOpType.is_ge
                )
                nc.scalar.dma_start(out=out[i * P:(i + 1) * P, :], in_=ot)

    _nc_cache = nc
    return nc


def kernel(inputs: np.ndarray, medians: np.ndarray) -> np.ndarray:
    global LAST_RESULTS
    inputs = np.ascontiguousarray(inputs, dtype=np.float32)
    medians = np.asarray(medians, dtype=np.float32)

    # Fold (medians > 0) into the threshold: anything with a non-positive
    # median compares against +inf, which no finite input reaches.
    thr = np.where(medians > 0.0, medians, np.float32(np.inf)).astype(np.float32)
    thr = thr.reshape(1, F)

    nc = _build_program()
    in_maps = [
        {"x": inputs[c * ROWS_PER_CORE:(c + 1) * ROWS_PER_CORE], "thr": thr}
        for c in range(NCORES)
    ]
    res = run_bass_kernel_spmd(
        nc, in_maps, core_ids=list(range(NCORES)), trace=TRACE
    )
    LAST_RESULTS = res

    out = np.empty((B, F), dtype=np.uint8)
    for c in range(NCORES):
        out[c * ROWS_PER_CORE:(c + 1) * ROWS_PER_CORE] = res.results[c]["out"]
    return out.view(np.bool_)
